# revision 1
# baseline (speedup 1.0000x reference)
"""Trainium2 Bass kernel: single-head causal self-attention.

reference:
    q = x @ Wq; k = x @ Wk; v = x @ Wv          (x: [B, T, 512], W*: [512, 64])
    scores = (q @ k^T) / sqrt(64), causal mask, softmax
    out = weights @ v                            -> [B, T, 64]

Strategy (8 NeuronCores, data-parallel over batch, 2 batches/core):
  - All matmuls in bf16 (fp32 PSUM accumulation); softmax in fp32 on ScalarE.
  - Transposed-scores layout scoresT[s, t]: softmax exp output (expT) feeds the
    PV matmul directly as the moving operand; no probability-matrix transpose.
  - Z (softmax denominator) comes free from a ones-column appended to V.
  - No max-subtraction: scores = q.k/8 with these scales are bounded (|s|<~4),
    exp is safe in fp32.
  - Causal masking: block skipping at 512-wide matmul granularity + 4
    precomputed multiplicative wedge masks for diagonal blocks.
  - x^T obtained via fp32->bf16 cast-DMA to DRAM scratch, then 2-byte DMA
    transposes (xbar) DRAM->SBUF.
"""

import numpy as np
import ml_dtypes

B, T, D, H = 16, 4096, 512, 64
N_CORES = 8
BPC = B // N_CORES  # batches per core

_BF16 = ml_dtypes.bfloat16


def build_nc(bpc=BPC, t=T, w=1024, mmw=512, reps=1, stage="full",
             pipe_a1=True, rch=None, interleave_a2=True, defer_fin=True,
             xtmode="dram", split_copies=True, pv_depth=3):
    """Build the per-core Bass program. All cores run the same program (SPMD).

    bpc: batches per core; t: sequence length; w: t-range (output tile) width;
    mmw: matmul moving-operand width (<= 512, psum bank granularity);
    reps: replicate the whole body (for wall-clock benchmarking).
    """
    import concourse.bacc as bacc
    import concourse.bass as bass
    import concourse.mybir as mybir
    import concourse.tile as tile
    from concourse.masks import make_identity

    fp32 = mybir.dt.float32
    bf16 = mybir.dt.bfloat16
    AF = mybir.ActivationFunctionType
    OP = mybir.AluOpType

    assert t % w == 0 and w % mmw == 0 and mmw % 128 == 0
    nmm = w // mmw            # matmul slices per t-range
    nch_total = t // 128      # total s-chunks
    ntb = w // 128            # 128-blocks per t-range
    ndc = D // 128            # contraction chunks for projections
    nmask = mmw // 128        # wedge mask variants
    PQ = 1024 if t >= 1024 else w   # proj psum tile width (t-slice)
    assert t % PQ == 0

    nc = bacc.Bacc(None, target_bir_lowering=False)

    xs = nc.dram_tensor("xs", [bpc, t, D], fp32, kind="ExternalInput")
    wqk = nc.dram_tensor("wqk", [D, 128], bf16, kind="ExternalInput")
    wv = nc.dram_tensor("wv", [D, H], bf16, kind="ExternalInput")
    mask4 = nc.dram_tensor("mask4", [nmask, 128, mmw], bf16, kind="ExternalInput")
    # staged layout [b, j, p, tb, h]: row t = j*w + tb*128 + p. Per-partition
    # contiguous 2KB runs (vs 256B strided for [t, h]) — host un-stages.
    out = nc.dram_tensor("out", [bpc, t // w, 128, ntb, H], fp32,
                         kind="ExternalOutput")

    xbf = nc.dram_tensor("xbf", [bpc, t, D], bf16)  # bf16 scratch for transposes

    with tile.TileContext(nc) as tc:
        with (
            tc.tile_pool(name="const", bufs=1) as cpool,
            tc.tile_pool(name="sb", bufs=1) as sb,
            tc.tile_pool(name="ps", bufs=1, space="PSUM") as ps,
        ):
            # ---- constants ----
            wqk_sb = cpool.tile([128, ndc, 128], bf16)
            nc.sync.dma_start(wqk_sb[:], wqk.rearrange("(c p) m -> p c m", p=128))
            wv_sb = cpool.tile([128, ndc, H], bf16)
            nc.sync.dma_start(wv_sb[:], wv.rearrange("(c p) m -> p c m", p=128))
            mask_sb = cpool.tile([128, nmask, mmw], bf16)
            nc.sync.dma_start(mask_sb[:], mask4.rearrange("k p f -> p k f"))
            ident = cpool.tile([128, 128], fp32)
            make_identity(nc, ident[:])
            # touch Exp once so the ~2.7us ACT table load happens during the
            # cold phase-A DMAs instead of before the first real exp
            warm = cpool.tile([1, 1], fp32)
            nc.scalar.activation(warm[:], ident[0:1, 0:1], AF.Exp)

            # row chunks for cast/transpose pipelining
            RCH = rch if rch is not None else max(1, t // 1024)

            def phase_a1(b):
                """DMA-only prologue: fp32->bf16 cast (SWDGE) + xbar
                DMA-transposes producing x^T chunks [128(d), t]. Split into
                row chunks so transposes pipeline behind the cast."""
                xt = [sb.tile([128, t], bf16, name=f"xt{dc}", tag=f"xt{dc}",
                              bufs=2)
                      for dc in range(ndc)]
                if xtmode == "sbuf":
                    # cast HBM->SBUF per row chunk, then 128x128 SBUF
                    # transposes; avoids the DRAM bf16 round trip.
                    rows = min(t, 1024)
                    for r in range(t // rows):
                        xn = sb.tile([128, rows // 128, D], bf16, name="xn",
                                     tag="xn", bufs=3)
                        nc.gpsimd.dma_start(
                            xn[:],
                            xs[b][r * rows:(r + 1) * rows, :].rearrange(
                                "(i p) d -> p i d", p=128),
                        )
                        for i in range(rows // 128):
                            t0 = r * rows + i * 128
                            for dc in range(ndc):
                                nc.sync.dma_start(
                                    xt[dc][:, t0:t0 + 128],
                                    xn[:, i, dc * 128:(dc + 1) * 128],
                                    transpose=True,
                                )
                    return xt
                for r in range(RCH):
                    rs = slice(r * (t // RCH), (r + 1) * (t // RCH))
                    nc.gpsimd.dma_start(xbf[b][rs], xs[b][rs])
                for dc in range(ndc):
                    for r in range(RCH):
                        rs = slice(r * (t // RCH), (r + 1) * (t // RCH))
                        nc.sync.dma_start(
                            xt[dc][:, rs], xbf[b][rs, dc * 128:(dc + 1) * 128],
                            transpose=True,
                        )
                return xt

            def phase_a2_units(b, xt):
                """Projections as deferred work units (one per t-slice of PQ,
                plus the v-transpose finisher). Returns (units, (qt, kt, vn));
                each unit is a callable emitting its instructions."""
                qt = sb.tile([64, t], bf16, name="qt", tag="qt", bufs=2)
                kt = sb.tile([64, t], bf16, name="kt", tag="kt", bufs=2)
                vt = sb.tile([64, t], bf16, name="vt", tag="vt", bufs=2)
                vn = sb.tile([128, t // 128, 80], bf16, name="vn", tag="vn",
                             bufs=2)

                def quarter_units(q):
                    hold = {}

                    def get_psum():
                        if "pqk" not in hold:
                            hold["pqk"] = ps.tile([128, PQ], fp32, name="pqk",
                                                  tag="sc", bufs=3)
                            hold["pv"] = ps.tile([64, PQ], fp32, name="pv",
                                                 tag="outp", bufs=1)
                        return hold["pqk"], hold["pv"]

                    def mms(s):
                        def emit():
                            pqk, pv = get_psum()
                            sl = slice(q * PQ + s * mmw,
                                       q * PQ + (s + 1) * mmw)
                            psl = slice(s * mmw, (s + 1) * mmw)
                            for dc in range(ndc):
                                nc.tensor.matmul(
                                    pqk[:, psl], wqk_sb[:, dc, :],
                                    xt[dc][:, sl],
                                    start=(dc == 0), stop=(dc == ndc - 1),
                                )
                            for dc in range(ndc):
                                nc.tensor.matmul(
                                    pv[:, psl], wv_sb[:, dc, :], xt[dc][:, sl],
                                    start=(dc == 0), stop=(dc == ndc - 1),
                                )
                        return emit

                    def copy_q():
                        pqk, _ = get_psum()
                        nc.vector.tensor_copy(
                            qt[:, q * PQ:(q + 1) * PQ], pqk[0:64, :])

                    def copy_k():
                        pqk, _ = get_psum()
                        nc.vector.tensor_copy(
                            kt[:, q * PQ:(q + 1) * PQ], pqk[64:128, :])

                    def copy_v():
                        _, pv = get_psum()
                        nc.vector.tensor_copy(
                            vt[:, q * PQ:(q + 1) * PQ], pv[:, :])

                    if split_copies:
                        return ([mms(s) for s in range(PQ // mmw)]
                                + [copy_q, copy_k, copy_v])

                    def copies():
                        copy_q(); copy_k(); copy_v()

                    return [mms(s) for s in range(PQ // mmw)] + [copies]

                def v_finish():
                    # v natural [s, h] with ones column at h=64 (pad stride 80)
                    nc.vector.memset(vn[:, :, 64:65], 1.0)
                    nc.sync.dma_start(vn[:, :, 0:64], vt[:], transpose=True)

                units = [u for q in range(t // PQ)
                         for u in quarter_units(q)] + [v_finish]
                return units, (qt, kt, vn)

            def phase_a2(b, xt):
                units, qkv = phase_a2_units(b, xt)
                for u in units:
                    u()
                return qkv

            def phase_b(b, qt, kt, vn, side_work=()):
                """Attention for batch b. side_work: deferred units (next
                batch's projections) interleaved into the c-loop so they hide
                under this batch's ScalarE-bound phase."""
                side = list(side_work)
                pending_fin = [None]  # deferred finalize closure
                pend_q = []  # PV queue; carried across t-ranges so the tail
                # of range j overlaps range j+1's score/exp ramp

                if stage == "phaseA":
                    nc.gpsimd.dma_start(out[b][0, :, 0, :],
                                        vn[:, 0, 0:64])  # keep outputs alive
                    return
                for j in range(t // w):
                    nch = min(nch_total, (j + 1) * w // 128)
                    outp = ps.tile([65, w], fp32, name="outp", tag="outp", bufs=1)

                    def emit_pv(c, et, los, outp=outp, nch=nch, j=j):
                        # default args bind THIS range's values: entries may be
                        # emitted during the next range's iteration
                        for h, lo in los:
                            c_last = min(nch - 1,
                                         (j * w + (h + 1) * mmw - 1) // 128)
                            nc.tensor.matmul(
                                outp[:, lo:(h + 1) * mmw],
                                vn[:, c, 0:65],
                                et[:, lo:(h + 1) * mmw],
                                start=(c == 0), stop=(c == c_last),
                            )

                    def finalize(j, outp):
                        # transpose [65, 128]->[128, 65] blocks, normalize by
                        # 1/Z (col 64), store staged output
                        osb = sb.tile([65, w], fp32, name="osb", tag="osb",
                                      bufs=2)
                        nc.vector.tensor_copy(osb[:], outp[:])
                        tp = ps.tile([128, ntb * 65], fp32, name="tp",
                                     tag="sc", bufs=3)
                        for tb in range(ntb):
                            nc.tensor.transpose(
                                tp[:, tb * 65:(tb + 1) * 65],
                                osb[:, tb * 128:(tb + 1) * 128],
                                ident[0:65, 0:65],
                            )
                        tpv = tp.rearrange("p (tb e) -> p tb e", e=65)
                        rz = sb.tile([128, ntb], fp32, name="rz", tag="rz",
                                     bufs=2)
                        nc.vector.reciprocal(rz[:], tpv[:, :, 64])
                        outf = sb.tile([128, ntb, H], fp32, name="outf",
                                       tag="outf", bufs=2)
                        rzb = rz[:].rearrange("p (n o) -> p n o",
                                              o=1).broadcast_to([128, ntb, H])
                        nc.vector.tensor_tensor(
                            outf[:], tpv[:, :, 0:64], rzb, op=OP.mult)
                        if stage == "nodma":
                            nc.sync.dma_start(out[b][j, :, 0, :],
                                              outf[:, 0, :])
                        else:
                            nc.sync.dma_start(out[b][j], outf[:])

                    # software-pipeline PV by pv_depth blocks: the PE stream is
                    # in-order, so PV(c) right after scores(c) would stall the
                    # PE on exp(c); emitting later scores first keeps the PE
                    # busy while the ScalarE computes exp(c).
                    for c in range(nch):
                        s0 = c * 128
                        diag = s0 >= j * w
                        off = s0 - j * w  # first causal col in this t-range
                        halves = [h for h in range(nmm)
                                  if j * w + (h + 1) * mmw > s0]
                        los = [(h, max(h * mmw, off) if diag else h * mmw)
                               for h in halves]

                        sc = ps.tile([128, w], fp32, name="sc", tag="sc", bufs=3)
                        for h, lo in los:
                            nc.tensor.matmul(
                                sc[:, lo:(h + 1) * mmw],
                                kt[:, s0:s0 + 128],
                                qt[:, j * w + lo: j * w + (h + 1) * mmw],
                                start=True, stop=True,
                            )
                        esl = slice(off if diag else 0, w)
                        et = sb.tile([128, w], bf16, name="et", tag="et", bufs=8)
                        nc.scalar.activation(et[:, esl], sc[:, esl], AF.Exp)
                        if diag:
                            # wedge corner [off, off+128): keep f-off >= p.
                            # GPSIMD (idle) keeps this off the DVE queue so PV
                            # isn't stalled behind DVE work.
                            nc.gpsimd.tensor_tensor(
                                et[:, off:off + 128], et[:, off:off + 128],
                                mask_sb[:, 0, 0:128], op=OP.mult,
                            )
                        if stage == "scores":
                            continue
                        if c == pv_depth and pending_fin[0] is not None:
                            # deferred finalize of the previous t-range: by
                            # c==pv_depth its PV tail has drained from pend_q,
                            # and exp(c=0..) keeps ACT busy over the transposes
                            pending_fin[0]()
                            pending_fin[0] = None
                        pend_q.append((emit_pv, (c, et, los)))
                        if len(pend_q) > pv_depth:
                            fn, args = pend_q.pop(0)
                            fn(*args)
                        if interleave_a2 and side and c >= 2:
                            side.pop(0)()  # next batch's projection slice
                    if j == t // w - 1 or stage == "pv":
                        while pend_q:
                            fn, args = pend_q.pop(0)
                            fn(*args)

                    # ---- finalize t-range (deferred into next j) ----
                    if stage == "scores":
                        # keep et alive via a dummy consumer DMA of one tile
                        if j == t // w - 1:
                            nc.gpsimd.dma_start(out[b][0, :, 0, :],
                                                et[0:128, 0:64])
                        continue
                    if stage == "pv":
                        # consume outp cheaply: copy one column out
                        dmy = sb.tile([65, 1], fp32, name="dmy", tag="dmy",
                                      bufs=2)
                        nc.vector.tensor_copy(dmy[:], outp[:, 0:1])
                        nc.sync.dma_start(out[b][j, 0:65, 0, 0:1], dmy[:])
                        continue
                    if pending_fin[0] is not None:
                        pending_fin[0]()
                        pending_fin[0] = None
                    if defer_fin:
                        pending_fin[0] = (lambda j=j, outp=outp:
                                          finalize(j, outp))
                    else:
                        finalize(j, outp)
                if stage in ("full", "nodma"):
                    if pending_fin[0] is not None:
                        pending_fin[0]()
                        pending_fin[0] = None
                for u in side:  # any side work not consumed inline
                    u()

            # Driver: software-pipeline phase A1 (pure DMA) one batch ahead so
            # batch i+1's HBM traffic hides under batch i's attention; batch
            # i+1's projections ride inside batch i's attention as side work.
            order = [bb for _ in range(reps) for bb in range(bpc)]
            if pipe_a1:
                xt_next = phase_a1(order[0])
                qkv = phase_a2(order[0], xt_next)
                for i, b in enumerate(order):
                    side = ()
                    if i + 1 < len(order):
                        xt_next = phase_a1(order[i + 1])
                        side, qkv_next = phase_a2_units(order[i + 1], xt_next)
                    phase_b(b, *qkv, side_work=side)
                    if i + 1 < len(order):
                        qkv = qkv_next
            else:
                for b in order:
                    xt = phase_a1(b)
                    qkv = phase_a2(b, xt)
                    phase_b(b, *qkv)
    nc.compile()
    return nc


def _host_inputs(Wq, Wk, Wv, t=T, mmw=512):
    """Host-side input prep shared by kernel() and tests."""
    nmask = mmw // 128
    scale = np.float32(1.0 / np.sqrt(H))
    wqk = np.concatenate([Wq * scale, Wk], axis=1).astype(_BF16)
    wv = Wv.astype(_BF16)
    masks = np.zeros((nmask, 128, mmw), dtype=_BF16)
    for k in range(nmask):
        p = np.arange(128)[:, None]
        f = np.arange(mmw)[None, :]
        masks[k] = (f >= p + 128 * k).astype(_BF16)
    return wqk, wv, masks


def unstage(staged, t=T):
    """[b, j, p, tb, h] staged device layout -> [b, t, h]."""
    b, nj, p, ntb, h = staged.shape
    return np.ascontiguousarray(
        staged.transpose(0, 1, 3, 2, 4).reshape(b, t, h))


def kernel(x, Wq, Wk, Wv):
    from concourse.bass_utils import run_bass_kernel_spmd

    x = np.asarray(x)
    wqk, wv, masks = _host_inputs(
        np.asarray(Wq, dtype=np.float32), np.asarray(Wk, dtype=np.float32),
        np.asarray(Wv, dtype=np.float32))

    nc = build_nc()
    core_ids = list(range(N_CORES))
    in_maps = [
        {
            "xs": np.ascontiguousarray(x[c * BPC:(c + 1) * BPC]),
            "wqk": wqk,
            "wv": wv,
            "mask4": masks,
        }
        for c in core_ids
    ]
    res = run_bass_kernel_spmd(nc, in_maps, core_ids)
    return np.concatenate(
        [unstage(res.results[c]["out"]) for c in core_ids], axis=0)



# revision 34
# speedup vs baseline: 1.5916x; 1.5916x over previous
"""Trainium2 Bass kernel: single-head causal self-attention.

reference:
    q = x @ Wq; k = x @ Wk; v = x @ Wv          (x: [B, T, 512], W*: [512, 64])
    scores = (q @ k^T) / sqrt(64), causal mask, softmax
    out = weights @ v                            -> [B, T, 64]

Strategy (8 NeuronCores, data-parallel over batch, 2 batches/core):
  - All matmuls in bf16 (fp32 PSUM accumulation); softmax in fp32 on ScalarE.
  - Transposed-scores layout scoresT[s, t]: softmax exp output (expT) feeds the
    PV matmuls directly; no probability-matrix transpose.
  - PV is et-stationary (pvmode="etst"): out^T blocks [128(t), 65] accumulate
    per 128-col slot of one [128, w] psum tile; moving operand is v-natural
    (65 cols incl. ones column for Z) — half the PE streaming of the
    vn-stationary form and no output transposes in finalize.
  - Z (softmax denominator) comes free from a ones-column appended to V.
  - No max-subtraction: scores = q.k/8 with these scales are bounded (|s|<~4),
    exp is safe in fp32.
  - Causal masking: block skipping at matmul granularity + precomputed
    multiplicative wedge masks for diagonal blocks (GPSIMD).
  - x^T via fp32->bf16 cast-DMA HBM->SBUF, then PE-transposes (xtmode="pe")
    emitted as interleavable side units; no DRAM round trip.
  - ScalarE (exp) is the bottleneck engine: score matmuls for chunk c+1 are
    emitted ahead of PV/side PE work (lookahead), and projection/transpose
    psum lives in a dedicated "aux" tag so side units never block the score
    ring ACT feeds from.
  - PSUM banks (8): sc ring 2x2 + outp 2 + aux 2.
"""

import numpy as np
import ml_dtypes

B, T, D, H = 16, 4096, 512, 64
N_CORES = 8
BPC = B // N_CORES  # batches per core

_BF16 = ml_dtypes.bfloat16


def build_nc(bpc=BPC, t=T, w=1024, mmw=512, reps=1, stage="full",
             pipe_a1=True, rch=None, interleave_a2=True, defer_fin=True,
             xtmode="pe", split_copies=True, pv_depth=3, pvmode="etst",
             rt=False, aux=True, lookahead=True, rows=1024):
    """Build the per-core Bass program. All cores run the same program (SPMD).

    bpc: batches per core; t: sequence length; w: t-range (output tile) width;
    mmw: matmul moving-operand width (<= 512, psum bank granularity);
    reps: replicate the whole body (for wall-clock benchmarking).
    """
    import concourse.bacc as bacc
    import concourse.bass as bass
    import concourse.mybir as mybir
    import concourse.tile as tile
    from concourse.masks import make_identity

    fp32 = mybir.dt.float32
    bf16 = mybir.dt.bfloat16
    AF = mybir.ActivationFunctionType
    OP = mybir.AluOpType

    assert t % w == 0 and w % mmw == 0 and mmw % 128 == 0
    nmm = w // mmw            # matmul slices per t-range
    nch_total = t // 128      # total s-chunks
    ntb = w // 128            # 128-blocks per t-range
    ndc = D // 128            # contraction chunks for projections
    nmask = mmw // 128        # wedge mask variants
    PQ = 1024 if t >= 1024 else w   # proj psum tile width (t-slice)
    assert t % PQ == 0

    nc = bacc.Bacc(None, target_bir_lowering=False)
    ROWS = min(t, rows)  # row-chunk granularity for x load/transpose

    xs = nc.dram_tensor("xs", [bpc, t, D], fp32, kind="ExternalInput")
    wqk = nc.dram_tensor("wqk", [D, 128], bf16, kind="ExternalInput")
    wv = nc.dram_tensor("wv", [D, H], bf16, kind="ExternalInput")
    mask4 = nc.dram_tensor("mask4", [nmask, 128, mmw], bf16, kind="ExternalInput")
    # staged layout [b, j, p, tb, h]: row t = j*w + tb*128 + p. Per-partition
    # contiguous 2KB runs (vs 256B strided for [t, h]) — host un-stages.
    out = nc.dram_tensor("out", [bpc, t // w, 128, ntb, H], fp32,
                         kind="ExternalOutput")

    xbf = nc.dram_tensor("xbf", [bpc, t, D], bf16)  # bf16 scratch for transposes

    with tile.TileContext(nc) as tc:
        with (
            tc.tile_pool(name="const", bufs=1) as cpool,
            tc.tile_pool(name="sb", bufs=1) as sb,
            tc.tile_pool(name="ps", bufs=1, space="PSUM") as ps,
        ):
            # ---- constants ----
            wqk_sb = cpool.tile([128, ndc, 128], bf16)
            nc.sync.dma_start(wqk_sb[:], wqk.rearrange("(c p) m -> p c m", p=128))
            wv_sb = cpool.tile([128, ndc, H], bf16)
            nc.sync.dma_start(wv_sb[:], wv.rearrange("(c p) m -> p c m", p=128))
            mask_sb = cpool.tile([128, nmask, mmw], bf16)
            nc.sync.dma_start(mask_sb[:], mask4.rearrange("k p f -> p k f"))
            ident = cpool.tile([128, 128], fp32)
            make_identity(nc, ident[:])
            identb = cpool.tile([128, 128], bf16)
            make_identity(nc, identb[:])
            # touch Exp once so the ~2.7us ACT table load happens during the
            # cold phase-A DMAs instead of before the first real exp
            warm = cpool.tile([1, 1], fp32)
            nc.scalar.activation(warm[:], ident[0:1, 0:1], AF.Exp)

            # row chunks for cast/transpose pipelining
            RCH = rch if rch is not None else max(1, t // 1024)

            def phase_a1(b):
                """x^T production. Returns (xt, units).

                xtmode="pe": emit only cast-DMAs (HBM fp32 -> SBUF bf16);
                return PE-transpose units (4 transposes + 1 wide evac per
                128-row block) to be interleaved as side work so they don't
                block the in-order PE queue behind in-flight DMAs.
                Other modes: emit everything, return no units."""
                if xtmode == "pe":
                    xt4 = sb.tile([128, ndc, t], bf16, name="xt4", tag="xt4",
                                  bufs=2)
                    xns = []
                    for r in range(t // ROWS):
                        xn = sb.tile([128, ROWS // 128, D], bf16, name="xn",
                                     tag="xn", bufs=3)
                        nc.gpsimd.dma_start(
                            xn[:],
                            xs[b][r * ROWS:(r + 1) * ROWS, :].rearrange(
                                "(i p) d -> p i d", p=128),
                        )
                        xns.append(xn)

                    def tunit(r, i, xn):
                        def emit():
                            t0 = r * ROWS + i * 128
                            tp = ps.tile([128, ndc, 128], bf16, name="xtp",
                                         tag=("aux" if aux else "xtp"),
                                         bufs=(1 if aux else 2))
                            for dc in range(ndc):
                                nc.tensor.transpose(
                                    tp[:, dc, :],
                                    xn[:, i, dc * 128:(dc + 1) * 128],
                                    identb[:],
                                )
                            nc.vector.tensor_copy(
                                xt4[:, :, t0:t0 + 128], tp[:])
                        return emit

                    units = [tunit(r, i, xns[r])
                             for r in range(t // ROWS)
                             for i in range(ROWS // 128)]
                    return xt4, units
                xt = [sb.tile([128, t], bf16, name=f"xt{dc}", tag=f"xt{dc}",
                              bufs=2)
                      for dc in range(ndc)]
                if xtmode == "sbuf":
                    # cast HBM->SBUF per row chunk, then 128x128 SBUF
                    # transposes; avoids the DRAM bf16 round trip.
                    rows = min(t, 1024)
                    for r in range(t // rows):
                        xn = sb.tile([128, rows // 128, D], bf16, name="xn",
                                     tag="xn", bufs=3)
                        nc.gpsimd.dma_start(
                            xn[:],
                            xs[b][r * rows:(r + 1) * rows, :].rearrange(
                                "(i p) d -> p i d", p=128),
                        )
                        for i in range(rows // 128):
                            t0 = r * rows + i * 128
                            for dc in range(ndc):
                                nc.sync.dma_start(
                                    xt[dc][:, t0:t0 + 128],
                                    xn[:, i, dc * 128:(dc + 1) * 128],
                                    transpose=True,
                                )
                    return xt, []
                for r in range(RCH):
                    rs = slice(r * (t // RCH), (r + 1) * (t // RCH))
                    nc.gpsimd.dma_start(xbf[b][rs], xs[b][rs])
                for dc in range(ndc):
                    for r in range(RCH):
                        rs = slice(r * (t // RCH), (r + 1) * (t // RCH))
                        nc.sync.dma_start(
                            xt[dc][:, rs], xbf[b][rs, dc * 128:(dc + 1) * 128],
                            transpose=True,
                        )
                return xt, []

            def phase_a2_units(b, xt):
                """Projections as deferred work units (one per t-slice of PQ,
                plus the v-transpose finisher). Returns (units, (qt, kt, vn));
                each unit is a callable emitting its instructions."""
                if xtmode == "pe":
                    xtsl = lambda dc, sl: xt[:, dc, sl]  # noqa: E731
                else:
                    xtsl = lambda dc, sl: xt[dc][:, sl]  # noqa: E731
                # rt: duplicate q/k onto partitions 64-127 so odd s-chunks'
                # score matmuls run on the second 64-row PE tile concurrently
                qkp = 128 if rt else 64
                qt = sb.tile([qkp, t], bf16, name="qt", tag="qt", bufs=2)
                kt = sb.tile([qkp, t], bf16, name="kt", tag="kt", bufs=2)
                vt = sb.tile([64, t], bf16, name="vt", tag="vt", bufs=2)
                vn = sb.tile([128, t // 128, 80], bf16, name="vn", tag="vn",
                             bufs=2)

                def quarter_units(q):
                    hold = {}

                    def get_pqk():
                        if "pqk" not in hold:
                            hold["pqk"] = ps.tile(
                                [128, PQ], fp32, name="pqk",
                                tag=("aux" if aux else "sc"),
                                bufs=(1 if aux else 2))
                        return hold["pqk"]

                    def get_pv():
                        if "pv" not in hold:
                            hold["pv"] = ps.tile(
                                [64, PQ], fp32, name="pv",
                                tag=("aux" if aux else "outp"), bufs=1)
                        return hold["pv"]

                    def mms_qk(s):
                        def emit():
                            pqk = get_pqk()
                            sl = slice(q * PQ + s * mmw,
                                       q * PQ + (s + 1) * mmw)
                            psl = slice(s * mmw, (s + 1) * mmw)
                            for dc in range(ndc):
                                nc.tensor.matmul(
                                    pqk[:, psl], wqk_sb[:, dc, :],
                                    xtsl(dc, sl),
                                    start=(dc == 0), stop=(dc == ndc - 1),
                                )
                        return emit

                    def mms_v(s):
                        def emit():
                            pv = get_pv()
                            sl = slice(q * PQ + s * mmw,
                                       q * PQ + (s + 1) * mmw)
                            psl = slice(s * mmw, (s + 1) * mmw)
                            for dc in range(ndc):
                                nc.tensor.matmul(
                                    pv[:, psl], wv_sb[:, dc, :], xtsl(dc, sl),
                                    start=(dc == 0), stop=(dc == ndc - 1),
                                )
                        return emit

                    def copy_q():
                        pqk = get_pqk()
                        nc.vector.tensor_copy(
                            qt[0:64, q * PQ:(q + 1) * PQ], pqk[0:64, :])
                        if rt:
                            nc.vector.tensor_copy(
                                qt[64:128, q * PQ:(q + 1) * PQ], pqk[0:64, :])

                    def copy_k():
                        pqk = get_pqk()
                        nc.vector.tensor_copy(
                            kt[0:64, q * PQ:(q + 1) * PQ], pqk[64:128, :])
                        if rt:
                            nc.vector.tensor_copy(
                                kt[64:128, q * PQ:(q + 1) * PQ], pqk[64:128, :])
                        hold.pop("pqk", None)  # release slot for next user

                    def copy_v():
                        pv = get_pv()
                        nc.vector.tensor_copy(
                            vt[:, q * PQ:(q + 1) * PQ], pv[:, :])
                        hold.pop("pv", None)

                    if aux:
                        # aux tag has a single slot: qk pass fully drains
                        # before the v pass claims it
                        return ([mms_qk(s) for s in range(PQ // mmw)]
                                + [copy_q, copy_k]
                                + [mms_v(s) for s in range(PQ // mmw)]
                                + [copy_v])

                    def mms(s):
                        qk, v = mms_qk(s), mms_v(s)

                        def emit():
                            qk()
                            v()
                        return emit

                    if split_copies:
                        return ([mms(s) for s in range(PQ // mmw)]
                                + [copy_q, copy_k, copy_v])

                    def copies():
                        copy_q(); copy_k(); copy_v()

                    return [mms(s) for s in range(PQ // mmw)] + [copies]

                def v_finish():
                    # v natural [s, h] with ones column at h=64 (pad stride 80)
                    nc.vector.memset(vn[:, :, 64:65], 1.0)
                    nc.sync.dma_start(vn[:, :, 0:64], vt[:], transpose=True)

                units = [u for q in range(t // PQ)
                         for u in quarter_units(q)] + [v_finish]
                return units, (qt, kt, vn)

            def merge_units(a1u, a2u):
                """Interleave x-transpose units (one per 128-row block) with
                projection units so each PQ-quarter's transposes precede its
                matmuls."""
                if not a1u:
                    return list(a2u)
                nq = t // PQ
                per_q = (len(a2u) - 1) // nq
                bpq = PQ // 128
                merged = []
                for q in range(nq):
                    merged += a1u[q * bpq:(q + 1) * bpq]
                    merged += a2u[q * per_q:(q + 1) * per_q]
                merged.append(a2u[-1])
                return merged

            def phase_b(b, qt, kt, vn, side_work=()):
                """Attention for batch b. side_work: deferred units (next
                batch's projections) interleaved into the c-loop so they hide
                under this batch's ScalarE-bound phase."""
                side = list(side_work)
                pending_fin = [None]  # deferred finalize closure
                pend_et = {}  # lookahead: exp outputs awaiting PV emission
                pend_q = []  # PV queue; carried across t-ranges so the tail
                # of range j overlaps range j+1's score/exp ramp
                gstep = 2 if rt else 1

                def score_group(j, g0):
                    """Emit score matmuls for chunks [g0, g0+gstep) of range
                    j; returns the group descriptors for later exp/PV."""
                    nch_j = min(nch_total, (j + 1) * w // 128)
                    group = []
                    for c in range(g0, min(g0 + gstep, nch_j)):
                        s0 = c * 128
                        diag = s0 >= j * w
                        off = s0 - j * w  # first causal col in this t-range
                        halves = [h for h in range(nmm)
                                  if j * w + (h + 1) * mmw > s0]
                        los = [(h, max(h * mmw, off) if diag else h * mmw)
                               for h in halves]

                        sc = ps.tile([128, w], fp32, name="sc", tag="sc",
                                     bufs=2)
                        hp = 64 * ((c - g0) % 2) if rt else 0
                        for h, lo in los:
                            nc.tensor.matmul(
                                sc[:, lo:(h + 1) * mmw],
                                kt[hp:hp + 64, s0:s0 + 128],
                                qt[hp:hp + 64,
                                   j * w + lo: j * w + (h + 1) * mmw],
                                start=True, stop=True,
                            )
                        group.append((c, sc, los, off, diag))
                    return group

                held = [None]  # next score group, possibly from range j+1

                if stage == "phaseA":
                    for u in side:  # still emit next batch's units
                        u()
                    nc.gpsimd.dma_start(out[b][0, :, 0, :],
                                        vn[:, 0, 0:64])  # keep outputs alive
                    return
                for j in range(t // w):
                    nch = min(nch_total, (j + 1) * w // 128)
                    if pvmode == "etst":
                        # et-stationary PV: out^T blocks [128(t), 65] at
                        # 128-col offsets in one [128, w] psum tile. Halves
                        # PV streaming (65 cols/pair vs 128) and kills the
                        # finalize transposes.
                        outp = ps.tile([128, w], fp32, name="outp",
                                       tag="outp", bufs=1)
                    else:
                        outp = ps.tile([65, w], fp32, name="outp", tag="outp",
                                       bufs=1)

                    def emit_pv(c, et, los, outp=outp, nch=nch, j=j):
                        # default args bind THIS range's values: entries may be
                        # emitted during the next range's iteration
                        if pvmode == "etst":
                            s0 = c * 128
                            tb_min = max(0, (s0 - j * w) // 128)
                            for tb in range(tb_min, ntb):
                                c_last = min(nch - 1, j * ntb + tb)
                                # start=True clears has_written for the WHOLE
                                # bank (4 tb blocks): emit it only on the
                                # first tb of each bank; later tb's first
                                # write overwrites per-element (bit clear)
                                nc.tensor.matmul(
                                    outp[:, tb * 128:tb * 128 + 65],
                                    et[:, tb * 128:(tb + 1) * 128],
                                    vn[:, c, 0:65],
                                    start=(c == 0 and tb % 4 == 0),
                                    stop=(c == c_last),
                                    skip_group_check=True,
                                )
                            return
                        for h, lo in los:
                            c_last = min(nch - 1,
                                         (j * w + (h + 1) * mmw - 1) // 128)
                            nc.tensor.matmul(
                                outp[:, lo:(h + 1) * mmw],
                                vn[:, c, 0:65],
                                et[:, lo:(h + 1) * mmw],
                                start=(c == 0), stop=(c == c_last),
                            )

                    def finalize(j, outp):
                        if pvmode == "etst":
                            # outp blocks already [t-part, 65]: recip z
                            # (col 64 of each block), scale, store
                            opv = outp.rearrange("p (tb e) -> p tb e", e=128)
                            rz = sb.tile([128, ntb], fp32, name="rz",
                                         tag="rz", bufs=2)
                            nc.vector.reciprocal(rz[:], opv[:, :, 64])
                            outf = sb.tile([128, ntb, H], fp32, name="outf",
                                           tag="outf", bufs=2)
                            rzb = rz[:].rearrange(
                                "p (n o) -> p n o",
                                o=1).broadcast_to([128, ntb, H])
                            nc.vector.tensor_tensor(
                                outf[:], opv[:, :, 0:64], rzb, op=OP.mult)
                            if stage == "nodma":
                                nc.sync.dma_start(out[b][j, :, 0, :],
                                                  outf[:, 0, :])
                            else:
                                nc.sync.dma_start(out[b][j], outf[:])
                            return
                        # transpose [65, 128]->[128, 65] blocks, normalize by
                        # 1/Z (col 64), store staged output
                        osb = sb.tile([65, w], fp32, name="osb", tag="osb",
                                      bufs=2)
                        nc.vector.tensor_copy(osb[:], outp[:])
                        tp = ps.tile([128, ntb * 65], fp32, name="tp",
                                     tag="sc", bufs=2)
                        for tb in range(ntb):
                            nc.tensor.transpose(
                                tp[:, tb * 65:(tb + 1) * 65],
                                osb[:, tb * 128:(tb + 1) * 128],
                                ident[0:65, 0:65],
                            )
                        tpv = tp.rearrange("p (tb e) -> p tb e", e=65)
                        rz = sb.tile([128, ntb], fp32, name="rz", tag="rz",
                                     bufs=2)
                        nc.vector.reciprocal(rz[:], tpv[:, :, 64])
                        outf = sb.tile([128, ntb, H], fp32, name="outf",
                                       tag="outf", bufs=2)
                        rzb = rz[:].rearrange("p (n o) -> p n o",
                                              o=1).broadcast_to([128, ntb, H])
                        nc.vector.tensor_tensor(
                            outf[:], tpv[:, :, 0:64], rzb, op=OP.mult)
                        if stage == "nodma":
                            nc.sync.dma_start(out[b][j, :, 0, :],
                                              outf[:, 0, :])
                        else:
                            nc.sync.dma_start(out[b][j], outf[:])

                    # software-pipeline PV by pv_depth blocks: the PE stream is
                    # in-order, so PV(c) right after scores(c) would stall the
                    # PE on exp(c); emitting later scores first keeps the PE
                    # busy while the ScalarE computes exp(c).
                    # rt: chunks are processed in pairs with both chunks'
                    # score matmuls emitted adjacently — even chunk on PE row
                    # tile (0,0), odd on (64,0), running concurrently.
                    if held[0] is None:
                        held[0] = score_group(j, 0)
                    for g0 in range(0, nch, gstep):
                        group = held[0]
                        held[0] = None
                        # emit this group's exps, then the NEXT group's score
                        # matmuls ahead of the PV/side PE work so the ACT
                        # stream never waits on scores stuck behind it
                        if lookahead:
                            for c, sc, los, off, diag in group:
                                esl = slice(off if diag else 0, w)
                                et = sb.tile([128, w], bf16, name="et",
                                             tag="et", bufs=8)
                                nc.scalar.activation(et[:, esl], sc[:, esl],
                                                     AF.Exp)
                                pend_et[(j, c)] = et
                            if g0 + gstep < nch:
                                held[0] = score_group(j, g0 + gstep)
                            elif j + 1 < t // w:
                                # cross-range lookahead: next range's first
                                # scores ride ahead of this range's PV tail
                                held[0] = score_group(j + 1, 0)
                        for c, sc, los, off, diag in group:
                            if lookahead:
                                et = pend_et.pop((j, c))
                            else:
                                esl = slice(off if diag else 0, w)
                                et = sb.tile([128, w], bf16, name="et",
                                             tag="et", bufs=8)
                                nc.scalar.activation(et[:, esl], sc[:, esl],
                                                     AF.Exp)
                            if diag:
                                # wedge corner [off, off+128): keep f-off >= p.
                                # GPSIMD (idle) keeps this off the DVE queue so
                                # PV isn't stalled behind DVE work.
                                nc.gpsimd.tensor_tensor(
                                    et[:, off:off + 128], et[:, off:off + 128],
                                    mask_sb[:, 0, 0:128], op=OP.mult,
                                )
                            if stage == "scores":
                                continue
                            if (c == pv_depth
                                    and pending_fin[0] is not None):
                                # deferred finalize of the previous t-range:
                                # by c==pv_depth its PV tail has drained, and
                                # exp keeps ACT busy over the transposes
                                pending_fin[0]()
                                pending_fin[0] = None
                            pend_q.append((emit_pv, (c, et, los)))
                            if len(pend_q) > pv_depth:
                                fn, args = pend_q.pop(0)
                                fn(*args)
                            if interleave_a2 and side and c >= 2:
                                side.pop(0)()  # next batch's proj slice
                        if not lookahead and g0 + gstep < nch:
                            held[0] = score_group(j, g0 + gstep)
                    if j == t // w - 1 or stage == "pv":
                        while pend_q:
                            fn, args = pend_q.pop(0)
                            fn(*args)

                    # ---- finalize t-range (deferred into next j) ----
                    if stage == "scores":
                        # keep et alive via a dummy consumer DMA of one tile
                        if j == t // w - 1:
                            nc.gpsimd.dma_start(out[b][0, :, 0, :],
                                                et[0:128, 0:64])
                        continue
                    if stage == "pv":
                        # consume outp cheaply: copy one column out
                        np_ = 128 if pvmode == "etst" else 65
                        dmy = sb.tile([np_, 1], fp32, name="dmy", tag="dmy",
                                      bufs=2)
                        nc.vector.tensor_copy(dmy[:], outp[:, 0:1])
                        nc.sync.dma_start(out[b][j, 0:np_, 0, 0:1], dmy[:])
                        continue
                    if pending_fin[0] is not None:
                        pending_fin[0]()
                        pending_fin[0] = None
                    if defer_fin:
                        pending_fin[0] = (lambda j=j, outp=outp:
                                          finalize(j, outp))
                    else:
                        finalize(j, outp)
                if stage in ("full", "nodma"):
                    if pending_fin[0] is not None:
                        pending_fin[0]()
                        pending_fin[0] = None
                for u in side:  # any side work not consumed inline
                    u()

            # Driver: software-pipeline phase A1 (pure DMA) one batch ahead so
            # batch i+1's HBM traffic hides under batch i's attention; batch
            # i+1's projections ride inside batch i's attention as side work.
            order = [bb for _ in range(reps) for bb in range(bpc)]
            if pipe_a1:
                xt_next, a1u = phase_a1(order[0])
                a2u, qkv = phase_a2_units(order[0], xt_next)
                for u in merge_units(a1u, a2u):
                    u()
                for i, b in enumerate(order):
                    side = ()
                    if i + 1 < len(order):
                        xt_next, a1u = phase_a1(order[i + 1])
                        a2u, qkv_next = phase_a2_units(order[i + 1], xt_next)
                        side = merge_units(a1u, a2u)
                    phase_b(b, *qkv, side_work=side)
                    if i + 1 < len(order):
                        qkv = qkv_next
            else:
                for b in order:
                    xt, a1u = phase_a1(b)
                    a2u, qkv = phase_a2_units(b, xt)
                    for u in merge_units(a1u, a2u):
                        u()
                    phase_b(b, *qkv)
    nc.compile()
    return nc


def _host_inputs(Wq, Wk, Wv, t=T, mmw=512):
    """Host-side input prep shared by kernel() and tests."""
    nmask = mmw // 128
    scale = np.float32(1.0 / np.sqrt(H))
    wqk = np.concatenate([Wq * scale, Wk], axis=1).astype(_BF16)
    wv = Wv.astype(_BF16)
    masks = np.zeros((nmask, 128, mmw), dtype=_BF16)
    for k in range(nmask):
        p = np.arange(128)[:, None]
        f = np.arange(mmw)[None, :]
        masks[k] = (f >= p + 128 * k).astype(_BF16)
    return wqk, wv, masks


def unstage(staged, t=T):
    """[b, j, p, tb, h] staged device layout -> [b, t, h]."""
    b, nj, p, ntb, h = staged.shape
    return np.ascontiguousarray(
        staged.transpose(0, 1, 3, 2, 4).reshape(b, t, h))


def kernel(x, Wq, Wk, Wv, **build_kwargs):
    from concourse.bass_utils import run_bass_kernel_spmd

    x = np.asarray(x)
    wqk, wv, masks = _host_inputs(
        np.asarray(Wq, dtype=np.float32), np.asarray(Wk, dtype=np.float32),
        np.asarray(Wv, dtype=np.float32))

    nc = build_nc(**build_kwargs)
    core_ids = list(range(N_CORES))
    in_maps = [
        {
            "xs": np.ascontiguousarray(x[c * BPC:(c + 1) * BPC]),
            "wqk": wqk,
            "wv": wv,
            "mask4": masks,
        }
        for c in core_ids
    ]
    res = run_bass_kernel_spmd(nc, in_maps, core_ids)
    return np.concatenate(
        [unstage(res.results[c]["out"]) for c in core_ids], axis=0)

